# revision 14
# baseline (speedup 1.0000x reference)
"""Per-pixel adaptive (kernel-prediction) 5x5 conv on 8 trn2 cores.

out[b,c,y,x] = sum_{i,j} x_pad[b,c,y+i,x+j] * kernel[b,(c*5+i)*5+j,y,x]
with edge (replication) padding p=2.

Sharding: 8 cores = B(4) x C-halves(2).  The op is depthwise (output
channel c reads only input channel c), so slicing C needs no halo.

Device layout: 128 SBUF partitions = 16 channels x 8 row-groups; each
partition owns a 36-row x 260-col stripe of padded x (halo included), so
every tap (i,j) is a strided view at free offset i*260+j.

The kernel tensor (the dominant HBM traffic, ~52MB/core as fp16) is
pre-permuted on the HOST into DMA-emission order: one fully-contiguous
HBM region per DMA group ([128 partitions x ntaps x 4096 px]), so the
16 SDMA engines stream sequential HBM instead of scattered 8KB chunks
(measured 240 -> ~430 GB/s).  Group sizes taper up at the start (so
compute starts at ~7us, not after a 5MB prefill) and down at the end
(short tail).  x is pre-strided into [128, 9360] and loaded first,
split across both HWDGE queues; kernel-group DMAs are balanced across
the two queues by byte count.  Output is stored in permuted [128, 8192]
layout that the host unpacks.

Per 16-row half-pass: DVE computes the 25 tap products (fp16, 2x mode),
and the otherwise-idle TensorE accumulates them into PSUM via identity
matmuls (PSUM accumulate-on-write does the adds for free).  ScalarE
drains PSUM to SBUF (DVE helps on the final drain); stores go out on
gpsimd mid-stream and on the idle HWDGE queues at the end.
"""

import numpy as np

B, C, H, W, K = 4, 32, 256, 256, 5
P = (K - 1) // 2  # 2
CP = 16           # channels per core
YG = 8            # row groups
RG = H // YG      # 32 rows per group
WP = W + 2 * P    # 260
SROWS = RG + 2 * P  # 36 rows per stripe
HR = RG // 2        # 16 rows per half-pass
XROWS = HR + 2 * P  # 20 rows per half-stripe (halo incl)
XLEN = XROWS * WP   # 5200 elems per partition half-stripe
HFREE = HR * W      # 4096 free elems per half-pass
HHALF = HFREE // 2  # 2048, psum drain chunk
NBANK = HFREE // 512  # 8 psum banks
GMAX = 5            # max taps per kernel-DMA group
GFREE = GMAX * HFREE

# (half, first tap, ntaps): taper up at start, down at the end
GROUPS = [
    (0, 0, 1), (0, 1, 2), (0, 3, 3), (0, 6, 4), (0, 10, 5), (0, 15, 5),
    (0, 20, 5),
    (1, 0, 5), (1, 5, 5), (1, 10, 5), (1, 15, 5), (1, 20, 4), (1, 24, 1),
]
KTOTAL = 128 * 2 * K * K * HFREE  # flat ks element count

_cache = {}


def _build_nc():
    import concourse.bass as bass
    import concourse.tile as tile
    from concourse import bacc, mybir

    f32 = mybir.dt.float32
    f16 = mybir.dt.float16
    nc = bacc.Bacc("TRN2", target_bir_lowering=False, debug=False, num_devices=8)

    xs_t = nc.dram_tensor("xs", [128, 2 * XLEN], f16, kind="ExternalInput")
    ks_t = nc.dram_tensor("ks", [KTOTAL], f16, kind="ExternalInput")
    ident_t = nc.dram_tensor("ident", [128, 128], f16, kind="ExternalInput")
    out_t = nc.dram_tensor("out", [128, 2 * HFREE], f16, kind="ExternalOutput")

    with tile.TileContext(nc) as tc:
        with (
            tc.tile_pool(name="xp", bufs=1) as xpool,
            tc.tile_pool(name="idp", bufs=1) as ipool,
            tc.tile_pool(name="kp", bufs=3) as kpool,
            tc.tile_pool(name="tp", bufs=6) as tpool,
            tc.tile_pool(name="op", bufs=2) as opool,
            tc.tile_pool(name="pp", bufs=1, space="PSUM") as ppool,
        ):
            # x half-stripes in separate tiles: the h=0 rows load first
            # (1.3MB, alone on sync while group 0 rides scalar) so the
            # first tap product starts as early as possible; the h=1
            # rows (with a 4-row halo overlap resent) follow.
            xtA = xpool.tile([128, XLEN], f16, tag="xa")
            xtB = xpool.tile([128, XLEN], f16, tag="xb")
            nc.sync.dma_start(out=xtA[:], in_=xs_t[:, :XLEN])
            nc.sync.dma_start(out=xtB[:], in_=xs_t[:, XLEN:])

            ident = ipool.tile([128, 128], f16)
            nc.gpsimd.dma_start(out=ident[:], in_=ident_t[:, :])

            x3h = [
                xtA[:].rearrange("p (r w) -> p r w", w=WP),
                xtB[:].rearrange("p (r w) -> p r w", w=WP),
            ]

            qbytes = {"sync": 2 * XLEN, "scalar": 0}  # x pre-charge
            off = 0
            for h, t0, nt in GROUPS:
                glen = nt * HFREE
                ktile = kpool.tile([128, GFREE], f16, tag="kt")
                ksrc = bass.AP(ks_t, off, [[glen, 128], [1, glen]])
                off += 128 * glen
                qname = min(qbytes, key=qbytes.get)
                qbytes[qname] += glen
                keng = nc.sync if qname == "sync" else nc.scalar
                keng.dma_start(out=ktile[:, :glen], in_=ksrc)

                if t0 == 0:
                    ptile = ppool.tile([128, HFREE], f32, tag="ps")
                for t in range(nt):
                    ij = t0 + t
                    i, j = divmod(ij, K)
                    k3 = ktile[:, t * HFREE : (t + 1) * HFREE].rearrange(
                        "p (r w) -> p r w", w=W
                    )
                    xv = x3h[h][:, i : i + HR, j : j + W]
                    tmp = tpool.tile([128, HFREE], f16, tag="tmp")
                    t3 = tmp[:].rearrange("p (r w) -> p r w", w=W)
                    nc.vector.tensor_mul(t3, xv, k3)
                    for bk in range(NBANK):
                        nc.tensor.matmul(
                            out=ptile[:, bk * 512 : (bk + 1) * 512],
                            lhsT=ident[:],
                            rhs=tmp[:, bk * 512 : (bk + 1) * 512],
                            start=(ij == 0),
                            stop=(ij == K * K - 1),
                        )

                if t0 + nt == K * K:  # end of half-pass: drain + store
                    for q in range(2):
                        ob = opool.tile([128, HHALF], f16, tag="ob")
                        src = ptile[:, q * HHALF : (q + 1) * HHALF]
                        if h == 1 and q == 1:
                            nc.vector.tensor_copy(ob[:], src)  # DVE idle now
                        else:
                            nc.scalar.copy(ob[:], src)
                        dst = bass.AP(
                            out_t,
                            h * HFREE + q * HHALF,
                            [[2 * HFREE, 128], [1, HHALF]],
                        )
                        if h == 0:
                            nc.gpsimd.dma_start(out=dst, in_=ob[:])
                        else:
                            seng = nc.sync if q == 0 else nc.scalar
                            seng.dma_start(out=dst, in_=ob[:])

    nc.compile()
    return nc


def _get_nc():
    if "nc" not in _cache:
        _cache["nc"] = _build_nc()
    return _cache["nc"]


_IDENT = np.eye(128, dtype=np.float16)

# row indices of each partition's half-stripes in the padded image:
# part A = stripe rows 0..19 (h=0), part B = stripe rows 16..35 (h=1)
_ROWIDXA = (np.arange(YG)[:, None] * RG + np.arange(XROWS)[None, :])
_ROWIDXB = _ROWIDXA + HR


def _make_in_maps(x, kernel):
    x = np.asarray(x, dtype=np.float32).astype(np.float16)
    kern = np.asarray(kernel, dtype=np.float32).astype(np.float16)
    xpad = np.pad(x, ((0, 0), (0, 0), (P, P), (P, P)), mode="edge")

    in_maps = []
    for core in range(8):
        b, half = divmod(core, 2)
        c0 = half * CP
        # x half-stripes: [16ch, 8grp, 20rows, 260] x 2 -> [128, 10400]
        xp_c = xpad[b, c0 : c0 + CP]
        xs = np.concatenate(
            [
                xp_c[:, _ROWIDXA, :].reshape(128, XLEN),
                xp_c[:, _ROWIDXB, :].reshape(128, XLEN),
            ],
            axis=1,
        )
        # [c, ij, g, h2, r, w] -> [h2, ij, c, g, r, w]
        kc = kern[b, c0 * K * K : (c0 + CP) * K * K]
        kc = kc.reshape(CP, K * K, YG, 2, HR, W).transpose(3, 1, 0, 2, 4, 5)
        ks = np.empty(KTOTAL, dtype=np.float16)
        off = 0
        for h, t0, nt in GROUPS:
            # region: [nt, c, g, r, w] -> [c, g, nt, r, w] -> [128, nt*4096]
            reg = kc[h, t0 : t0 + nt].transpose(1, 2, 0, 3, 4)
            n = 128 * nt * HFREE
            ks[off : off + n] = reg.reshape(-1)
            off += n
        in_maps.append(
            {"xs": np.ascontiguousarray(xs), "ks": ks, "ident": _IDENT}
        )
    return in_maps


def kernel(x, kernel, kernel_size):
    from concourse.bass_utils import run_bass_kernel_spmd

    in_maps = _make_in_maps(x, kernel)
    nc = _get_nc()
    res = run_bass_kernel_spmd(nc, in_maps, list(range(8)))

    out = np.empty((B, C, H, W), dtype=np.float32)
    for core in range(8):
        b, half = divmod(core, 2)
        c0 = half * CP
        # [128, 8192] = [c, g, h, r, w] with y = g*32 + h*16 + r
        o = res.results[core]["out"].reshape(CP, YG, 2, HR, W)
        out[b, c0 : c0 + CP] = o.reshape(CP, H, W).astype(np.float32)
    return out
